# revision 8
# baseline (speedup 1.0000x reference)
"""GroupWhitening1d Trainium2 kernel.

x: [16384, 4096] f32, G=32 groups of d=128.
  out = (x - mean) @ blockdiag(W_g),  W_g = U_g S_g^-1/2 U_g^T from eigh of
  per-group covariance.

Strategy (data-parallel over rows, 8 cores x 2048 rows):
  Host pre-casts x to fp8(e3m4) and fp16 shards.
  K1 (device): Pool queue streams fp8 row-tiles into per-group Gram matmuls
      (PE, f32 PSUM, all 8 banks); concurrently the sync queue XBAR-
      transpose-loads the fp16 columns of groups 0..15 into PERSISTENT
      per-group SBUF tensors in x^T layout (partition = within-group
      feature d, free = row n).  NOTE: concurrent XBAR transposes on two
      different HWDGE queues corrupt each other on real TRN2, so all
      transposes within a kernel stay on one queue.
  Host: reduce grams over cores (f64), cov via fp8 means, eigh (f64), W;
      pack W_g blocks (fp16, partition = d) and per-feature bias
      b = -(mu W) as per-partition scalars.
  K2 (device): sync queue transpose-loads groups 16..31 into the same
      persistent cache while the whitening matmuls (W_g stationary, f32
      PSUM [128,1024] chunks) run over the already-cached groups; the
      PSUM evacuation (f32->f16 + centering bias) alternates DVE/Act —
      the two engines that can read PSUM — and stores go out on
      pool/sync/scalar.  Host transposes out^T back and casts to f32.
"""

import sys
import numpy as np

if "/opt/trn_rl_repo" not in sys.path:
    sys.path.insert(0, "/opt/trn_rl_repo")

N, D, G, d = 16384, 4096, 32, 128
NCORES = 8
NS = N // NCORES  # rows per core
NT = NS // 128  # row tiles per core
GK1 = 16  # groups transpose-cached by K1 (rest by K2)

_built = {}


def _sched(weights, n):
    """Deterministic weighted round-robin schedule of length n."""
    accum = dict.fromkeys(weights, 0.0)
    total = sum(weights.values())
    out = []
    for _ in range(n):
        for k in accum:
            accum[k] += weights[k] / total
        pick = max(accum, key=lambda kk: accum[kk])
        accum[pick] -= 1
        out.append(pick)
    return out


# K2 evacuation engines per [128,1024] chunk (2 per group, 64 total;
# GPSIMD cannot access PSUM so only DVE/Act qualify) and store queues.
K2_EVAC = _sched({"v": 1, "s": 1}, 64)
K2_STQ = _sched({"gpsimd": 20, "sync": 8, "scalar": 4}, 32)


def _alloc_cache(nc, mybir):
    """Per-group persistent x^T tensors; separate tensors kill WAW dep
    chains between the XBAR transpose writes. Must be allocated in the
    same order in K1/K2 so addresses line up."""
    return [
        nc.alloc_sbuf_tensor(f"xtc{g:02d}", [128, NS], mybir.dt.float16)
        for g in range(G)
    ]


def _build_k1(ns=NS):
    from concourse import bacc, mybir, tile

    f8, f32 = mybir.dt.float8e3, mybir.dt.float32
    f16 = mybir.dt.float16
    nc = bacc.Bacc(None, target_bir_lowering=False)
    x8 = nc.dram_tensor("x8", [ns, D], f8, kind="ExternalInput")
    xh = nc.dram_tensor("xh", [ns, D], f16, kind="ExternalInput")
    # layout [bank, d, gsub, e]; host: reshape/transpose to [G,d,d]
    gram = nc.dram_tensor("gram", [8, 128, 512], f32, kind="ExternalOutput")
    cache = _alloc_cache(nc, mybir)
    with tile.TileContext(nc) as tc:
        with (
            tc.tile_pool(name="ld", bufs=4) as ld,
            tc.tile_pool(name="ev", bufs=8) as ev,
            tc.tile_pool(name="ps", bufs=8, space="PSUM") as ps,
        ):
            gp = [
                ps.tile([128, 512], f32, tag="gram", name=f"gram{b}")
                for b in range(8)
            ]
            for t in range(NT):
                # XBAR transpose-loads of groups 0..GK1-1, all on sync
                # (one per tile iteration -> 16 total)
                if t < GK1:
                    nc.sync.dma_start_transpose(
                        cache[t].ap()[:], xh[:, t * 128:(t + 1) * 128]
                    )
                # fp8 row tile for the gram matmuls
                x8t = ld.tile([128, D], f8, tag="x8")
                nc.gpsimd.dma_start(x8t[:], x8[t * 128:(t + 1) * 128, :])
                for g in range(G):
                    b, s = divmod(g, 4)
                    xg = x8t[:, g * 128:(g + 1) * 128]
                    # one accumulation group per PSUM bank: start zeroes the
                    # whole zero region, so only the first slice starts
                    nc.tensor.matmul(
                        gp[b][:, s * 128:(s + 1) * 128],
                        xg,
                        xg,
                        start=(t == 0 and s == 0),
                        stop=(t == NT - 1 and s == 3),
                    )
            for b in range(8):
                e = ev.tile([128, 512], f32, tag="ev")
                if b % 2 == 0:
                    nc.vector.tensor_copy(e[:], gp[b][:])
                else:
                    nc.scalar.activation(
                        e[:], gp[b][:], mybir.ActivationFunctionType.Copy
                    )
                q = (nc.sync, nc.scalar, nc.gpsimd)[b % 3]
                q.dma_start(gram[b], e[:])
    nc.compile()
    return nc


def _build_k2(ns=NS):
    from concourse import bacc, mybir, tile

    f16, f32 = mybir.dt.float16, mybir.dt.float32
    nc = bacc.Bacc(None, target_bir_lowering=False)
    # W_g stationary blocks: wp[:, g*128:(g+1)*128] = W_g (partition = d)
    wp = nc.dram_tensor("wp", [128, D], f16, kind="ExternalInput")
    xh = nc.dram_tensor("xh", [ns, D], f16, kind="ExternalInput")
    # per-feature bias as per-partition scalars: bb[f, g] = -(mu_g W_g)[f]
    bb = nc.dram_tensor("bb", [128, G], f32, kind="ExternalInput")
    # out^T: rows = feature (g*128+f), cols = n
    outT = nc.dram_tensor("outT", [D, ns], f16, kind="ExternalOutput")
    # must match _build_k1's allocations exactly (same names/shapes/order)
    cache = _alloc_cache(nc, mybir)
    with tile.TileContext(nc) as tc:
        with (
            tc.tile_pool(name="cp", bufs=1) as cp,
            tc.tile_pool(name="st", bufs=4) as st,
            tc.tile_pool(name="pq", bufs=4, space="PSUM") as pq,
        ):
            # transpose-load the groups K1 didn't cache (sync queue only)
            for g in range(GK1, G):
                nc.sync.dma_start_transpose(
                    cache[g].ap()[:], xh[:, g * 128:(g + 1) * 128]
                )
            wps = cp.tile([128, D], f16, tag="wp")
            for c in range(4):
                q = nc.scalar if c % 2 == 0 else nc.gpsimd
                q.dma_start(
                    wps[:, c * 1024:(c + 1) * 1024],
                    wp[:, c * 1024:(c + 1) * 1024],
                )
            bbs = cp.tile([128, G], f32, tag="bb")
            nc.scalar.dma_start(bbs[:], bb[:])
            for g in range(G):
                o = st.tile([128, ns], f16, tag="st")
                bg = bbs[:, g:g + 1]
                for h in range(2):  # [128,1024] f32 PSUM halves (2 banks)
                    p = pq.tile([128, 1024], f32, tag="pq")
                    for c in range(2):
                        cc = h * 1024 + c * 512
                        nc.tensor.matmul(
                            p[:, c * 512:(c + 1) * 512],
                            wps[:, g * 128:(g + 1) * 128],
                            cache[g].ap()[:, cc:cc + 512],
                            start=True,
                            stop=True,
                        )
                    # evacuation converts f32->f16 and adds the centering bias
                    sl = slice(h * 1024, (h + 1) * 1024)
                    if K2_EVAC[2 * g + h] == "v":
                        nc.vector.tensor_scalar_add(o[:, sl], p[:], bg)
                    else:
                        nc.scalar.add(o[:, sl], p[:], bg)
                getattr(nc, K2_STQ[g]).dma_start(
                    outT[g * 128:(g + 1) * 128, :], o[:]
                )
    nc.compile()
    return nc


def _sbuf_addrs(nc):
    out = {}
    for a in nc.m.functions[0].allocations:
        if hasattr(a, "memorylocations") and a.memorylocations:
            ml = a.memorylocations[0]
            if ml.name.startswith("xtc"):
                out[ml.name] = getattr(ml, "addr", None)
    return out


def _host_solve(gram, mu8):
    """gram: [G,d,d] f64 raw sum of q8(x)_g^T q8(x)_g; mu8: [D] f64."""
    mug = mu8.reshape(G, d)
    cov = (gram - N * np.einsum("gd,ge->gde", mug, mug)) / (N - 1)
    cov = (cov + cov.transpose(0, 2, 1)) / 2
    S, U = np.linalg.eigh(cov)
    S = np.maximum(S, 1e-12)
    W = np.einsum("gde,ge,gfe->gdf", U, 1.0 / np.sqrt(S), U)
    return W  # [G, d, d]


def kernel(x):
    import ml_dtypes
    from concourse.bass_utils import run_bass_kernel_spmd

    x = np.ascontiguousarray(x, dtype=np.float32)
    core_ids = list(range(NCORES))
    xh = x.astype(np.float16)
    x8 = x.astype(ml_dtypes.float8_e3m4)

    if "k1" not in _built:
        _built["k1"] = _build_k1()
    if "k2" not in _built:
        _built["k2"] = _build_k2()
        a1 = _sbuf_addrs(_built["k1"])
        a2 = _sbuf_addrs(_built["k2"])
        assert a1 == a2 and len(a1) == G, (a1, a2)

    in1 = [
        {
            "x8": x8[c * NS:(c + 1) * NS],
            "xh": xh[c * NS:(c + 1) * NS],
        }
        for c in range(NCORES)
    ]
    r1 = run_bass_kernel_spmd(_built["k1"], in1, core_ids)
    gram = np.zeros((G, d, d), np.float64)
    for r in r1.results:
        # [8, 128, 512] -> [8, 128, 4, 128] -> [8, 4, 128, 128] -> [G, d, d]
        gram += (
            r["gram"].astype(np.float64)
            .reshape(8, 128, 4, 128)
            .transpose(0, 2, 1, 3)
            .reshape(G, d, d)
        )

    mu8 = x8.astype(np.float64).mean(axis=0)
    W = _host_solve(gram, mu8)

    # wp[:, g*128:(g+1)*128] = W_g with partition = d (W symmetric)
    wpk = np.ascontiguousarray(
        W.transpose(1, 0, 2).reshape(d, D).astype(np.float16)
    )
    mu64 = x.mean(axis=0, dtype=np.float64)
    bvec = -np.einsum("gd,gdf->gf", mu64.reshape(G, d), W)  # [G, d]
    bbb = np.ascontiguousarray(bvec.T.astype(np.float32))  # [d, G]

    in2 = [
        {
            "wp": wpk,
            "bb": bbb,
            "xh": xh[c * NS:(c + 1) * NS],
        }
        for c in range(NCORES)
    ]
    global _last_in2
    _last_in2 = in2
    r2 = run_bass_kernel_spmd(_built["k2"], in2, core_ids)
    return np.concatenate(
        [r["outT"].T.astype(np.float32) for r in r2.results], axis=0
    )


# revision 19
# speedup vs baseline: 1.9844x; 1.9844x over previous
"""GroupWhitening1d Trainium2 kernel.

x: [16384, 4096] f32, G=32 groups of d=128.
  out = (x - mean) @ blockdiag(W_g),  W_g = U_g S_g^-1/2 U_g^T from eigh of
  per-group covariance.

Strategy (data-parallel over rows, 8 cores x 2048 rows):
  K1 (device): fp16 row tiles stream from HBM on all 3 DMA rings
      (SP/Act/Pool) directly into a PERSISTENT SBUF row cache; per-group
      Gram matmuls (PE, f32 PSUM, all 8 banks) read the cache slices.
  Host: reduce grams over cores (f64), cov, eigh (f64), W; pack W_g
      blocks (fp16, partition = d) and per-feature bias b = -(mu W) as
      per-partition scalars.
  K2 (device): zero input traffic -- software-pipelined by one group:
      16 PE transposes flip the cached row-major [128,128] blocks of
      group g+1 into x^T form (f16 PSUM, [128,2048] = 2 banks) while DVE
      evacuates group g's transposes to SBUF staging and the whitening
      matmuls (W_g stationary, f32 PSUM [128,1024]) run over group g-1;
      the f32->f16 whitening evacuation adds the centering bias (DVE/Act
      split -- the only engines that can read PSUM), stores go out on
      sync/pool rings.  Host transposes out^T back and casts to f32.

  (A DMA-engine XBAR transpose-load was tried instead of PE transposes:
  concurrent XBAR streams on two HWDGE queues corrupt each other on real
  TRN2, and the tile scheduler serializes every neighboring DMA against
  an InstDmaTransposeAnt, so the XBAR path cannot be made both safe and
  fast here.)
"""

import sys
import numpy as np

if "/opt/trn_rl_repo" not in sys.path:
    sys.path.insert(0, "/opt/trn_rl_repo")

N, D, G, d = 16384, 4096, 32, 128
NCORES = 8
NS = N // NCORES  # rows per core
NT = NS // 128  # row tiles per core

_built = {}


def _sched(weights, n):
    """Deterministic weighted round-robin schedule of length n."""
    accum = dict.fromkeys(weights, 0.0)
    total = sum(weights.values())
    out = []
    for _ in range(n):
        for k in accum:
            accum[k] += weights[k] / total
        pick = max(accum, key=lambda kk: accum[kk])
        accum[pick] -= 1
        out.append(pick)
    return out


# K1 load ring per row tile / K2 store ring per group (stores stay off
# the Act ring: the Act engine is saturated by whitening evacuation)
K1_LDQ = _sched({"sync": 6, "scalar": 5, "gpsimd": 5}, NT)
K2_STQ = _sched({"gpsimd": 17, "sync": 15}, G)
# K2 whitening-evac engine per [128,1024] f32 chunk (2 per group):
# DVE also carries all transpose evacs, so Act takes most of these
K2_WEVAC = _sched({"s": 52, "v": 12}, 64)


def _build_k1(ns=NS):
    from concourse import bacc, mybir, tile

    f16, f32 = mybir.dt.float16, mybir.dt.float32
    nc = bacc.Bacc(None, target_bir_lowering=False)
    xh = nc.dram_tensor("xh", [ns, D], f16, kind="ExternalInput")
    # layout [bank, d, gsub, e]; host: reshape/transpose to [G,d,d]
    gram = nc.dram_tensor("gram", [8, 128, 512], f32, kind="ExternalOutput")
    # persistent row cache: tile t at cols [t*D, (t+1)*D)
    cache = nc.alloc_sbuf_tensor("xrc", [128, NT * D], f16)
    with tile.TileContext(nc) as tc:
        with (
            tc.tile_pool(name="ev", bufs=8) as ev,
            tc.tile_pool(name="ps", bufs=8, space="PSUM") as ps,
        ):
            gp = [
                ps.tile([128, 512], f32, tag="gram", name=f"gram{b}")
                for b in range(8)
            ]
            for t in range(NT):
                if t == 0:
                    # split the first tile across rings so the PE can start
                    # on group 0 after ~1/4 of the tile has landed
                    for c in range(4):
                        q = (nc.sync, nc.scalar, nc.gpsimd, nc.sync)[c]
                        q.dma_start(
                            cache.ap()[:, c * 1024:(c + 1) * 1024],
                            xh[0:128, c * 1024:(c + 1) * 1024],
                        )
                else:
                    csl = cache.ap()[:, t * D:(t + 1) * D]
                    getattr(nc, K1_LDQ[t]).dma_start(
                        csl, xh[t * 128:(t + 1) * 128, :]
                    )
                for g in range(G):
                    b, s = divmod(g, 4)
                    xg = cache.ap()[:, t * D + g * 128: t * D + (g + 1) * 128]
                    # one accumulation group per PSUM bank: start zeroes the
                    # whole zero region, so only the first slice starts
                    nc.tensor.matmul(
                        gp[b][:, s * 128:(s + 1) * 128],
                        xg,
                        xg,
                        start=(t == 0 and s == 0),
                        stop=(t == NT - 1 and s == 3),
                    )
            for b in range(8):
                e = ev.tile([128, 512], f32, tag="ev")
                if b % 2 == 0:
                    nc.vector.tensor_copy(e[:], gp[b][:])
                else:
                    nc.scalar.activation(
                        e[:], gp[b][:], mybir.ActivationFunctionType.Copy
                    )
                q = (nc.sync, nc.scalar, nc.gpsimd)[b % 3]
                q.dma_start(gram[b], e[:])
    nc.compile()
    return nc


def _build_k2(ns=NS):
    from concourse import bacc, mybir, tile

    f16, f32 = mybir.dt.float16, mybir.dt.float32
    nc = bacc.Bacc(None, target_bir_lowering=False)
    # W_g stationary blocks: wp[:, g*128:(g+1)*128] = W_g (partition = d)
    wp = nc.dram_tensor("wp", [128, D], f16, kind="ExternalInput")
    idn = nc.dram_tensor("idn", [128, 128], f16, kind="ExternalInput")
    # per-feature bias as per-partition scalars: bb[f, g] = -(mu_g W_g)[f]
    bb = nc.dram_tensor("bb", [128, G], f32, kind="ExternalInput")
    # out^T: rows = feature (g*128+f), cols = n
    outT = nc.dram_tensor("outT", [D, ns], f16, kind="ExternalOutput")
    # must match _build_k1's allocation exactly (same name/shape/order)
    cache = nc.alloc_sbuf_tensor("xrc", [128, NT * D], f16)
    with tile.TileContext(nc) as tc:
        with (
            tc.tile_pool(name="cp", bufs=1) as cp,
            tc.tile_pool(name="xs", bufs=3) as xs,
            tc.tile_pool(name="st", bufs=4) as st,
            tc.tile_pool(name="pt", bufs=2, space="PSUM") as pt,
            tc.tile_pool(name="pw", bufs=2, space="PSUM") as pw,
        ):
            # identity first: the transposes only need ids + the resident
            # cache, so they start while the W chunks are still loading
            ids = cp.tile([128, 128], f16, tag="idn")
            nc.scalar.dma_start(ids[:], idn[:])
            wps = cp.tile([128, D], f16, tag="wp")
            for c in range(4):
                q = (nc.sync, nc.gpsimd, nc.sync, nc.gpsimd)[c]
                q.dma_start(
                    wps[:, c * 1024:(c + 1) * 1024],
                    wp[:, c * 1024:(c + 1) * 1024],
                )
            bbs = cp.tile([128, G], f32, tag="bb")
            nc.scalar.dma_start(bbs[:], bb[:])

            # software-pipelined by one group: the PE queue is in-order, so
            # transpose(g+1) is issued before whiten(g) -- the PE works on
            # g+1's transposes while DVE stages g's x^T
            xts = {}

            def emit_xpose(g):
                # PE-transpose the 16 cached row-major [128,128] blocks of
                # this group into x^T [d, n] form, staged via f16 PSUM
                # ([128,2048] f16 = 2 banks; zero regions start per bank)
                xt = xs.tile([128, ns], f16, tag="xt")
                ptile = pt.tile([128, 2048], f16, tag="pt")
                for t in range(NT):
                    nc.tensor.matmul(
                        ptile[:, t * 128:(t + 1) * 128],
                        cache.ap()[:, t * D + g * 128: t * D + (g + 1) * 128],
                        ids[:],
                        is_transpose=True,
                        start=(t % 8 == 0),
                        stop=(t % 8 == 7),
                    )
                nc.vector.tensor_copy(xt[:], ptile[:])
                xts[g] = xt

            def emit_whiten(g):
                xt = xts.pop(g)
                o = st.tile([128, ns], f16, tag="st")
                bg = bbs[:, g:g + 1]
                for h in range(2):
                    p = pw.tile([128, 1024], f32, tag="pw")
                    for c in range(2):
                        cc = h * 1024 + c * 512
                        nc.tensor.matmul(
                            p[:, c * 512:(c + 1) * 512],
                            wps[:, g * 128:(g + 1) * 128],
                            xt[:, cc:cc + 512],
                            start=True,
                            stop=True,
                        )
                    # evacuation converts f32->f16 and adds the centering bias
                    sl = slice(h * 1024, (h + 1) * 1024)
                    if K2_WEVAC[2 * g + h] == "v":
                        nc.vector.tensor_scalar_add(o[:, sl], p[:], bg)
                    else:
                        nc.scalar.add(o[:, sl], p[:], bg)
                getattr(nc, K2_STQ[g]).dma_start(
                    outT[g * 128:(g + 1) * 128, :], o[:]
                )

            emit_xpose(0)
            for g in range(1, G):
                emit_xpose(g)
                emit_whiten(g - 1)
            emit_whiten(G - 1)
    nc.compile()
    return nc


def _sbuf_addr(nc, name):
    for a in nc.m.functions[0].allocations:
        if hasattr(a, "memorylocations") and a.memorylocations:
            ml = a.memorylocations[0]
            if ml.name == name:
                return getattr(ml, "addr", None)
    return None


def _host_solve(gram, mu):
    """gram: [G,d,d] f64 raw sum of q16(x)_g^T q16(x)_g; mu: [D] f64."""
    mug = mu.reshape(G, d)
    cov = (gram - N * np.einsum("gd,ge->gde", mug, mug)) / (N - 1)
    cov = (cov + cov.transpose(0, 2, 1)) / 2
    S, U = np.linalg.eigh(cov)
    S = np.maximum(S, 1e-12)
    W = np.einsum("gde,ge,gfe->gdf", U, 1.0 / np.sqrt(S), U)
    return W  # [G, d, d]


def kernel(x):
    from concourse.bass_utils import run_bass_kernel_spmd

    x = np.ascontiguousarray(x, dtype=np.float32)
    core_ids = list(range(NCORES))
    xh = x.astype(np.float16)

    if "k1" not in _built:
        _built["k1"] = _build_k1()
    if "k2" not in _built:
        _built["k2"] = _build_k2()
        a1 = _sbuf_addr(_built["k1"], "xrc")
        a2 = _sbuf_addr(_built["k2"], "xrc")
        assert a1 == a2 and a1 is not None, (a1, a2)

    in1 = [{"xh": xh[c * NS:(c + 1) * NS]} for c in range(NCORES)]
    r1 = run_bass_kernel_spmd(_built["k1"], in1, core_ids)
    gram = np.zeros((G, d, d), np.float64)
    for r in r1.results:
        # [8, 128, 512] -> [8, 128, 4, 128] -> [8, 4, 128, 128] -> [G, d, d]
        gram += (
            r["gram"].astype(np.float64)
            .reshape(8, 128, 4, 128)
            .transpose(0, 2, 1, 3)
            .reshape(G, d, d)
        )

    mu16 = xh.astype(np.float64).mean(axis=0)
    W = _host_solve(gram, mu16)

    # wp[:, g*128:(g+1)*128] = W_g with partition = d (W symmetric)
    wpk = np.ascontiguousarray(
        W.transpose(1, 0, 2).reshape(d, D).astype(np.float16)
    )
    mu64 = x.mean(axis=0, dtype=np.float64)
    bvec = -np.einsum("gd,gdf->gf", mu64.reshape(G, d), W)  # [G, d]
    bbb = np.ascontiguousarray(bvec.T.astype(np.float32))  # [d, G]
    idn = np.eye(128, dtype=np.float16)

    in2 = [{"wp": wpk, "bb": bbb, "idn": idn} for _ in range(NCORES)]
    global _last_in2
    _last_in2 = in2
    r2 = run_bass_kernel_spmd(_built["k2"], in2, core_ids)
    return np.concatenate(
        [r["outT"].T.astype(np.float32) for r in r2.results], axis=0
    )


# revision 24
# speedup vs baseline: 2.0228x; 1.0194x over previous
"""GroupWhitening1d Trainium2 kernel.

x: [16384, 4096] f32, G=32 groups of d=128.
  out = (x - mean) @ blockdiag(W_g),  W_g = U_g S_g^-1/2 U_g^T from eigh of
  per-group covariance.

Strategy (data-parallel over rows, 8 cores x 2048 rows):
  K1 (device): fp16 row tiles stream from HBM on all 3 DMA rings
      (SP/Act/Pool) directly into a PERSISTENT SBUF row cache; per-group
      Gram matmuls (PE, f32 PSUM, all 8 banks) read the cache slices.
  Host: reduce grams over cores (f64), cov, eigh (f64), W; pack W_g
      blocks (fp16, partition = d) and per-feature bias b = -(mu W) as
      per-partition scalars.
  K2 (device): zero input traffic -- software-pipelined by one group:
      16 PE transposes flip the cached row-major [128,128] blocks of
      group g+1 into x^T form (f16 PSUM, [128,2048] = 2 banks) while DVE
      evacuates group g's transposes to SBUF staging and the whitening
      matmuls (W_g stationary, f32 PSUM [128,1024]) run over group g-1;
      the f32->f16 whitening evacuation adds the centering bias (DVE/Act
      split -- the only engines that can read PSUM), stores go out on
      sync/pool rings.  Host transposes out^T back and casts to f32.

  (A DMA-engine XBAR transpose-load was tried instead of PE transposes:
  concurrent XBAR streams on two HWDGE queues corrupt each other on real
  TRN2, and the tile scheduler serializes every neighboring DMA against
  an InstDmaTransposeAnt, so the XBAR path cannot be made both safe and
  fast here.)
"""

import sys
import numpy as np

if "/opt/trn_rl_repo" not in sys.path:
    sys.path.insert(0, "/opt/trn_rl_repo")

N, D, G, d = 16384, 4096, 32, 128
NCORES = 8
NS = N // NCORES  # rows per core
NT = NS // 128  # row tiles per core

_built = {}


def _sched(weights, n):
    """Deterministic weighted round-robin schedule of length n."""
    accum = dict.fromkeys(weights, 0.0)
    total = sum(weights.values())
    out = []
    for _ in range(n):
        for k in accum:
            accum[k] += weights[k] / total
        pick = max(accum, key=lambda kk: accum[kk])
        accum[pick] -= 1
        out.append(pick)
    return out


# K1 load ring per row tile / K2 store ring per group (stores stay off
# the Act ring: the Act engine is saturated by whitening evacuation)
K1_LDQ = _sched({"sync": 6, "scalar": 5, "gpsimd": 5}, NT)
K2_STQ = _sched({"gpsimd": 17, "sync": 15}, G)
# K2 whitening-evac engine per [128,1024] f32 chunk (2 per group):
# DVE also carries all transpose evacs, so Act takes most of these.
# The final group's two chunks go to different engines so the kernel
# tail evacuates in parallel.
K2_WEVAC = _sched({"s": 52, "v": 12}, 64)
K2_WEVAC[-2:] = ["v", "s"]


def _build_k1(ns=NS):
    from concourse import bacc, mybir, tile

    f16, f32 = mybir.dt.float16, mybir.dt.float32
    nc = bacc.Bacc(None, target_bir_lowering=False)
    xh = nc.dram_tensor("xh", [ns, D], f16, kind="ExternalInput")
    # layout [bank, d, gsub, e]; host: reshape/transpose to [G,d,d]
    gram = nc.dram_tensor("gram", [8, 128, 512], f32, kind="ExternalOutput")
    # persistent row cache: tile t at cols [t*D, (t+1)*D)
    cache = nc.alloc_sbuf_tensor("xrc", [128, NT * D], f16)
    with tile.TileContext(nc) as tc:
        with (
            tc.tile_pool(name="ev", bufs=8) as ev,
            tc.tile_pool(name="ps", bufs=8, space="PSUM") as ps,
        ):
            gp = [
                ps.tile([128, 512], f32, tag="gram", name=f"gram{b}")
                for b in range(8)
            ]
            # PE p-state warmup: dummy self-contained matmuls keep the PE
            # continuously busy through the first-load head so it reaches
            # full clock when the real gram stream starts (start=True
            # zeroes the bank again for the real accumulation group)
            z = ev.tile([128, 128], f16, tag="warm")
            nc.vector.memset(z[:], 0.0)
            for _ in range(60):
                nc.tensor.matmul(
                    gp[0][:, 0:1], z[:], z[:, 0:1], start=True, stop=True
                )
            for t in range(NT):
                if t == 0:
                    # split the first tile across rings so the PE can start
                    # on group 0 after ~1/4 of the tile has landed
                    for c in range(4):
                        q = (nc.sync, nc.scalar, nc.gpsimd, nc.sync)[c]
                        q.dma_start(
                            cache.ap()[:, c * 1024:(c + 1) * 1024],
                            xh[0:128, c * 1024:(c + 1) * 1024],
                        )
                else:
                    csl = cache.ap()[:, t * D:(t + 1) * D]
                    getattr(nc, K1_LDQ[t]).dma_start(
                        csl, xh[t * 128:(t + 1) * 128, :]
                    )
                for g in range(G):
                    b, s = divmod(g, 4)
                    xg = cache.ap()[:, t * D + g * 128: t * D + (g + 1) * 128]
                    # one accumulation group per PSUM bank: start zeroes the
                    # whole zero region, so only the first slice starts
                    nc.tensor.matmul(
                        gp[b][:, s * 128:(s + 1) * 128],
                        xg,
                        xg,
                        start=(t == 0 and s == 0),
                        stop=(t == NT - 1 and s == 3),
                    )
            for b in range(8):
                e = ev.tile([128, 512], f32, tag="ev")
                if b % 2 == 0:
                    nc.vector.tensor_copy(e[:], gp[b][:])
                else:
                    nc.scalar.activation(
                        e[:], gp[b][:], mybir.ActivationFunctionType.Copy
                    )
                q = (nc.sync, nc.scalar, nc.gpsimd)[b % 3]
                q.dma_start(gram[b], e[:])
    nc.compile()
    return nc


def _build_k2(ns=NS):
    from concourse import bacc, mybir, tile

    f16, f32 = mybir.dt.float16, mybir.dt.float32
    nc = bacc.Bacc(None, target_bir_lowering=False)
    # W_g stationary blocks: wp[:, g*128:(g+1)*128] = W_g (partition = d)
    wp = nc.dram_tensor("wp", [128, D], f16, kind="ExternalInput")
    idn = nc.dram_tensor("idn", [128, 128], f16, kind="ExternalInput")
    # per-feature bias as per-partition scalars: bb[f, g] = -(mu_g W_g)[f]
    bb = nc.dram_tensor("bb", [128, G], f32, kind="ExternalInput")
    # out^T: rows = feature (g*128+f), cols = n
    outT = nc.dram_tensor("outT", [D, ns], f16, kind="ExternalOutput")
    # must match _build_k1's allocation exactly (same name/shape/order)
    cache = nc.alloc_sbuf_tensor("xrc", [128, NT * D], f16)
    with tile.TileContext(nc) as tc:
        with (
            tc.tile_pool(name="cp", bufs=1) as cp,
            tc.tile_pool(name="xs", bufs=3) as xs,
            tc.tile_pool(name="st", bufs=4) as st,
            tc.tile_pool(name="pt", bufs=2, space="PSUM") as pt,
            tc.tile_pool(name="pw", bufs=2, space="PSUM") as pw,
        ):
            # identity first: the transposes only need ids + the resident
            # cache, so they start while the W chunks are still loading
            ids = cp.tile([128, 128], f16, tag="idn")
            nc.scalar.dma_start(ids[:], idn[:])
            wps = cp.tile([128, D], f16, tag="wp")
            for c in range(4):
                q = (nc.sync, nc.gpsimd, nc.sync, nc.gpsimd)[c]
                q.dma_start(
                    wps[:, c * 1024:(c + 1) * 1024],
                    wp[:, c * 1024:(c + 1) * 1024],
                )
            bbs = cp.tile([128, G], f32, tag="bb")
            nc.scalar.dma_start(bbs[:], bb[:])

            # PE p-state warmup through the ids/W load head (see K1)
            z = cp.tile([128, 128], f16, tag="warm")
            nc.vector.memset(z[:], 0.0)
            pz = pw.tile([128, 1024], f32, tag="pw")
            for _ in range(50):
                nc.tensor.matmul(
                    pz[:, 0:1], z[:], z[:, 0:1], start=True, stop=True
                )

            # software-pipelined by one group: the PE queue is in-order, so
            # transpose(g+1) is issued before whiten(g) -- the PE works on
            # g+1's transposes while DVE stages g's x^T
            xts = {}

            def emit_xpose(g):
                # PE-transpose the 16 cached row-major [128,128] blocks of
                # this group into x^T [d, n] form, staged via f16 PSUM
                # ([128,2048] f16 = 2 banks; zero regions start per bank)
                xt = xs.tile([128, ns], f16, tag="xt")
                ptile = pt.tile([128, 2048], f16, tag="pt")
                for t in range(NT):
                    nc.tensor.matmul(
                        ptile[:, t * 128:(t + 1) * 128],
                        cache.ap()[:, t * D + g * 128: t * D + (g + 1) * 128],
                        ids[:],
                        is_transpose=True,
                        start=(t % 8 == 0),
                        stop=(t % 8 == 7),
                    )
                nc.vector.tensor_copy(xt[:], ptile[:])
                xts[g] = xt

            def emit_whiten(g):
                xt = xts.pop(g)
                o = st.tile([128, ns], f16, tag="st")
                bg = bbs[:, g:g + 1]
                for h in range(2):
                    p = pw.tile([128, 1024], f32, tag="pw")
                    for c in range(2):
                        cc = h * 1024 + c * 512
                        nc.tensor.matmul(
                            p[:, c * 512:(c + 1) * 512],
                            wps[:, g * 128:(g + 1) * 128],
                            xt[:, cc:cc + 512],
                            start=True,
                            stop=True,
                        )
                    # evacuation converts f32->f16 and adds the centering bias
                    sl = slice(h * 1024, (h + 1) * 1024)
                    if K2_WEVAC[2 * g + h] == "v":
                        nc.vector.tensor_scalar_add(o[:, sl], p[:], bg)
                    else:
                        nc.scalar.add(o[:, sl], p[:], bg)
                if g == G - 1:
                    # split the last store across two rings: halves the
                    # kernel's store tail
                    half = ns // 2
                    nc.sync.dma_start(
                        outT[g * 128:(g + 1) * 128, 0:half], o[:, 0:half]
                    )
                    nc.gpsimd.dma_start(
                        outT[g * 128:(g + 1) * 128, half:ns], o[:, half:ns]
                    )
                else:
                    getattr(nc, K2_STQ[g]).dma_start(
                        outT[g * 128:(g + 1) * 128, :], o[:]
                    )

            emit_xpose(0)
            for g in range(1, G):
                emit_xpose(g)
                emit_whiten(g - 1)
            emit_whiten(G - 1)
    nc.compile()
    return nc


def _sbuf_addr(nc, name):
    for a in nc.m.functions[0].allocations:
        if hasattr(a, "memorylocations") and a.memorylocations:
            ml = a.memorylocations[0]
            if ml.name == name:
                return getattr(ml, "addr", None)
    return None


def _host_solve(gram, mu):
    """gram: [G,d,d] f64 raw sum of q16(x)_g^T q16(x)_g; mu: [D] f64."""
    mug = mu.reshape(G, d)
    cov = (gram - N * np.einsum("gd,ge->gde", mug, mug)) / (N - 1)
    cov = (cov + cov.transpose(0, 2, 1)) / 2
    S, U = np.linalg.eigh(cov)
    S = np.maximum(S, 1e-12)
    W = np.einsum("gde,ge,gfe->gdf", U, 1.0 / np.sqrt(S), U)
    return W  # [G, d, d]


def kernel(x):
    from concourse.bass_utils import run_bass_kernel_spmd

    x = np.ascontiguousarray(x, dtype=np.float32)
    core_ids = list(range(NCORES))
    xh = x.astype(np.float16)

    if "k1" not in _built:
        _built["k1"] = _build_k1()
    if "k2" not in _built:
        _built["k2"] = _build_k2()
        a1 = _sbuf_addr(_built["k1"], "xrc")
        a2 = _sbuf_addr(_built["k2"], "xrc")
        assert a1 == a2 and a1 is not None, (a1, a2)

    in1 = [{"xh": xh[c * NS:(c + 1) * NS]} for c in range(NCORES)]
    r1 = run_bass_kernel_spmd(_built["k1"], in1, core_ids)
    gram = np.zeros((G, d, d), np.float64)
    for r in r1.results:
        # [8, 128, 512] -> [8, 128, 4, 128] -> [8, 4, 128, 128] -> [G, d, d]
        gram += (
            r["gram"].astype(np.float64)
            .reshape(8, 128, 4, 128)
            .transpose(0, 2, 1, 3)
            .reshape(G, d, d)
        )

    mu16 = xh.astype(np.float64).mean(axis=0)
    W = _host_solve(gram, mu16)

    # wp[:, g*128:(g+1)*128] = W_g with partition = d (W symmetric)
    wpk = np.ascontiguousarray(
        W.transpose(1, 0, 2).reshape(d, D).astype(np.float16)
    )
    mu64 = x.mean(axis=0, dtype=np.float64)
    bvec = -np.einsum("gd,gdf->gf", mu64.reshape(G, d), W)  # [G, d]
    bbb = np.ascontiguousarray(bvec.T.astype(np.float32))  # [d, G]
    idn = np.eye(128, dtype=np.float16)

    in2 = [{"wp": wpk, "bb": bbb, "idn": idn} for _ in range(NCORES)]
    global _last_in2
    _last_in2 = in2
    r2 = run_bass_kernel_spmd(_built["k2"], in2, core_ids)
    return np.concatenate(
        [r["outT"].T.astype(np.float32) for r in r2.results], axis=0
    )


# revision 33
# speedup vs baseline: 2.1265x; 1.0512x over previous
"""GroupWhitening1d Trainium2 kernel.

x: [16384, 4096] f32, G=32 groups of d=128.
  out = (x - mean) @ blockdiag(W_g),  W_g = U_g S_g^-1/2 U_g^T from eigh of
  per-group covariance.

Strategy (data-parallel over rows, 8 cores x 2048 rows):
  K1 (device): fp16 row tiles stream from HBM on all 3 DMA rings
      (SP/Act/Pool) directly into a PERSISTENT SBUF row cache; per-group
      Gram matmuls (PE, f32 PSUM, all 8 banks) read the cache slices.
  Host: reduce grams over cores (f64), cov, eigh (f64), W; pack W_g
      blocks (fp16, partition = d) and per-feature bias b = -(mu W) as
      per-partition scalars.
  K2 (device): zero input traffic -- software-pipelined by one group:
      16 PE transposes flip the cached row-major [128,128] blocks of
      group g+1 into x^T form (f16 PSUM, [128,2048] = 2 banks) while DVE
      evacuates group g's transposes to SBUF staging and the whitening
      matmuls (W_g stationary, f32 PSUM [128,1024]) run over group g-1;
      the f32->f16 whitening evacuation adds the centering bias (DVE/Act
      split -- the only engines that can read PSUM), stores go out on
      sync/pool rings.  Host transposes out^T back and casts to f32.

  (A DMA-engine XBAR transpose-load was tried instead of PE transposes:
  concurrent XBAR streams on two HWDGE queues corrupt each other on real
  TRN2, and the tile scheduler serializes every neighboring DMA against
  an InstDmaTransposeAnt, so the XBAR path cannot be made both safe and
  fast here.)
"""

import sys
import numpy as np

if "/opt/trn_rl_repo" not in sys.path:
    sys.path.insert(0, "/opt/trn_rl_repo")

N, D, G, d = 16384, 4096, 32, 128
NCORES = 8
NS = N // NCORES  # rows per core
NT = NS // 128  # row tiles per core

_built = {}


def _sched(weights, n):
    """Deterministic weighted round-robin schedule of length n."""
    accum = dict.fromkeys(weights, 0.0)
    total = sum(weights.values())
    out = []
    for _ in range(n):
        for k in accum:
            accum[k] += weights[k] / total
        pick = max(accum, key=lambda kk: accum[kk])
        accum[pick] -= 1
        out.append(pick)
    return out


# K1 load ring per row tile / K2 store ring per group (stores stay off
# the Act ring: the Act engine is saturated by whitening evacuation)
K1_LDQ = _sched({"sync": 6, "scalar": 5, "gpsimd": 5}, NT)
K2_STQ = _sched({"gpsimd": 17, "sync": 15}, G)
# K2 whitening-evac engine per [128,1024] f32 chunk (2 per group):
# DVE also carries all transpose evacs, so Act takes most of these.
# The final group's two chunks go to different engines so the kernel
# tail evacuates in parallel.
K2_WEVAC = _sched({"s": 56, "v": 8}, 64)
K2_WEVAC[-2:] = ["v", "s"]


def _build_k1(ns=NS):
    from concourse import bacc, mybir, tile

    f16, f32 = mybir.dt.float16, mybir.dt.float32
    nc = bacc.Bacc(None, target_bir_lowering=False)
    xh = nc.dram_tensor("xh", [ns, D], f16, kind="ExternalInput")
    # layout [bank, d, gsub, e]; host: reshape/transpose to [G,d,d]
    gram = nc.dram_tensor("gram", [8, 128, 512], f32, kind="ExternalOutput")
    # persistent row cache: tile t at cols [t*D, (t+1)*D)
    cache = nc.alloc_sbuf_tensor("xrc", [128, NT * D], f16)
    with tile.TileContext(nc) as tc:
        with (
            tc.tile_pool(name="ev", bufs=8) as ev,
            tc.tile_pool(name="ps", bufs=8, space="PSUM") as ps,
        ):
            gp = [
                ps.tile([128, 512], f32, tag="gram", name=f"gram{b}")
                for b in range(8)
            ]
            # PE p-state warmup: dummy self-contained matmuls keep the PE
            # continuously busy through the first-load head so it reaches
            # full clock when the real gram stream starts (start=True
            # zeroes the bank again for the real accumulation group)
            z = ev.tile([128, 128], f16, tag="warm")
            nc.vector.memset(z[:], 0.0)
            for _ in range(60):
                nc.tensor.matmul(
                    gp[0][:, 0:1], z[:], z[:, 0:1], start=True, stop=True
                )
            for t in range(NT):
                if t == 0:
                    # split the first tile across rings so the PE can start
                    # on group 0 after ~1/4 of the tile has landed
                    for c in range(4):
                        q = (nc.sync, nc.scalar, nc.gpsimd, nc.sync)[c]
                        q.dma_start(
                            cache.ap()[:, c * 1024:(c + 1) * 1024],
                            xh[0:128, c * 1024:(c + 1) * 1024],
                        )
                else:
                    csl = cache.ap()[:, t * D:(t + 1) * D]
                    getattr(nc, K1_LDQ[t]).dma_start(
                        csl, xh[t * 128:(t + 1) * 128, :]
                    )
                for g in range(G):
                    b, s = divmod(g, 4)
                    xg = cache.ap()[:, t * D + g * 128: t * D + (g + 1) * 128]
                    # one accumulation group per PSUM bank: start zeroes the
                    # whole zero region, so only the first slice starts
                    nc.tensor.matmul(
                        gp[b][:, s * 128:(s + 1) * 128],
                        xg,
                        xg,
                        start=(t == 0 and s == 0),
                        stop=(t == NT - 1 and s == 3),
                    )
            for b in range(8):
                e = ev.tile([128, 512], f32, tag="ev")
                if b % 2 == 0:
                    nc.vector.tensor_copy(e[:], gp[b][:])
                else:
                    nc.scalar.activation(
                        e[:], gp[b][:], mybir.ActivationFunctionType.Copy
                    )
                q = (nc.sync, nc.scalar, nc.gpsimd)[b % 3]
                q.dma_start(gram[b], e[:])
    nc.compile()
    return nc


def _build_k2(ns=NS):
    from concourse import bacc, mybir, tile

    f16, f32 = mybir.dt.float16, mybir.dt.float32
    nc = bacc.Bacc(None, target_bir_lowering=False)
    # W_g stationary blocks: wp[:, g*128:(g+1)*128] = W_g (partition = d)
    wp = nc.dram_tensor("wp", [128, D], f16, kind="ExternalInput")
    idn = nc.dram_tensor("idn", [128, 128], f16, kind="ExternalInput")
    # per-feature bias as per-partition scalars: bb[f, g] = -(mu_g W_g)[f]
    bb = nc.dram_tensor("bb", [128, G], f32, kind="ExternalInput")
    # out^T: rows = feature (g*128+f), cols = n
    outT = nc.dram_tensor("outT", [D, ns], f16, kind="ExternalOutput")
    # must match _build_k1's allocation exactly (same name/shape/order)
    cache = nc.alloc_sbuf_tensor("xrc", [128, NT * D], f16)
    with tile.TileContext(nc) as tc:
        with (
            tc.tile_pool(name="cp", bufs=1) as cp,
            tc.tile_pool(name="xs", bufs=3) as xs,
            tc.tile_pool(name="st", bufs=4) as st,
            tc.tile_pool(name="pt", bufs=2, space="PSUM") as pt,
            tc.tile_pool(name="pw", bufs=3, space="PSUM") as pw,
        ):
            # identity first: the transposes only need ids + the resident
            # cache, so they start while the W chunks are still loading
            ids = cp.tile([128, 128], f16, tag="idn")
            nc.scalar.dma_start(ids[:], idn[:])
            wps = cp.tile([128, D], f16, tag="wp")
            for c in range(4):
                q = (nc.sync, nc.gpsimd, nc.sync, nc.gpsimd)[c]
                q.dma_start(
                    wps[:, c * 1024:(c + 1) * 1024],
                    wp[:, c * 1024:(c + 1) * 1024],
                )
            bbs = cp.tile([128, G], f32, tag="bb")
            nc.scalar.dma_start(bbs[:], bb[:])

            # PE p-state warmup through the ids/W load head (see K1)
            z = cp.tile([128, 128], f16, tag="warm")
            nc.vector.memset(z[:], 0.0)
            pz = pw.tile([128, 1024], f32, tag="pw")
            for _ in range(50):
                nc.tensor.matmul(
                    pz[:, 0:1], z[:], z[:, 0:1], start=True, stop=True
                )

            # software-pipelined by one group: the PE queue is in-order, so
            # transpose(g+1) is issued before whiten(g) -- the PE works on
            # g+1's transposes while DVE stages g's x^T
            xts = {}

            def emit_xpose(g):
                # PE-transpose the 16 cached row-major [128,128] blocks of
                # this group into x^T [d, n] form, staged via f16 PSUM
                # ([128,2048] f16 = 2 banks; zero regions start per bank)
                xt = xs.tile([128, ns], f16, tag="xt")
                for hh in range(2):
                    ptile = pt.tile([128, 1024], f16, tag="pt")
                    for k in range(8):
                        t = hh * 8 + k
                        nc.tensor.matmul(
                            ptile[:, k * 128:(k + 1) * 128],
                            cache.ap()[:, t * D + g * 128:
                                       t * D + (g + 1) * 128],
                            ids[:],
                            is_transpose=True,
                            start=(k == 0),
                            stop=(k == 7),
                        )
                    # per-half evacuation starts staging the first half
                    # while the PE transposes the second
                    nc.vector.tensor_copy(
                        xt[:, hh * 1024:(hh + 1) * 1024], ptile[:]
                    )
                xts[g] = xt

            def emit_whiten(g):
                xt = xts.pop(g)
                o = st.tile([128, ns], f16, tag="st")
                bg = bbs[:, g:g + 1]
                for h in range(2):
                    p = pw.tile([128, 1024], f32, tag="pw")
                    for c in range(2):
                        cc = h * 1024 + c * 512
                        nc.tensor.matmul(
                            p[:, c * 512:(c + 1) * 512],
                            wps[:, g * 128:(g + 1) * 128],
                            xt[:, cc:cc + 512],
                            start=True,
                            stop=True,
                        )
                    # evacuation converts f32->f16 and adds the centering bias
                    sl = slice(h * 1024, (h + 1) * 1024)
                    if K2_WEVAC[2 * g + h] == "v":
                        nc.vector.tensor_scalar_add(o[:, sl], p[:], bg)
                    else:
                        nc.scalar.add(o[:, sl], p[:], bg)
                if g == G - 1:
                    # split the last store across two rings: halves the
                    # kernel's store tail
                    half = ns // 2
                    nc.sync.dma_start(
                        outT[g * 128:(g + 1) * 128, 0:half], o[:, 0:half]
                    )
                    nc.gpsimd.dma_start(
                        outT[g * 128:(g + 1) * 128, half:ns], o[:, half:ns]
                    )
                else:
                    getattr(nc, K2_STQ[g]).dma_start(
                        outT[g * 128:(g + 1) * 128, :], o[:]
                    )

            emit_xpose(0)
            for g in range(1, G):
                emit_xpose(g)
                emit_whiten(g - 1)
            emit_whiten(G - 1)
    nc.compile()
    return nc


def _sbuf_addr(nc, name):
    for a in nc.m.functions[0].allocations:
        if hasattr(a, "memorylocations") and a.memorylocations:
            ml = a.memorylocations[0]
            if ml.name == name:
                return getattr(ml, "addr", None)
    return None


def _host_solve(gram, mu):
    """gram: [G,d,d] f64 raw sum of q16(x)_g^T q16(x)_g; mu: [D] f64."""
    mug = mu.reshape(G, d)
    cov = (gram - N * np.einsum("gd,ge->gde", mug, mug)) / (N - 1)
    cov = (cov + cov.transpose(0, 2, 1)) / 2
    S, U = np.linalg.eigh(cov)
    S = np.maximum(S, 1e-12)
    W = np.einsum("gde,ge,gfe->gdf", U, 1.0 / np.sqrt(S), U)
    return W  # [G, d, d]


def kernel(x):
    from concourse.bass_utils import run_bass_kernel_spmd

    x = np.ascontiguousarray(x, dtype=np.float32)
    core_ids = list(range(NCORES))
    xh = x.astype(np.float16)

    if "k1" not in _built:
        _built["k1"] = _build_k1()
    if "k2" not in _built:
        _built["k2"] = _build_k2()
        a1 = _sbuf_addr(_built["k1"], "xrc")
        a2 = _sbuf_addr(_built["k2"], "xrc")
        assert a1 == a2 and a1 is not None, (a1, a2)

    in1 = [{"xh": xh[c * NS:(c + 1) * NS]} for c in range(NCORES)]
    r1 = run_bass_kernel_spmd(_built["k1"], in1, core_ids)
    gram = np.zeros((G, d, d), np.float64)
    for r in r1.results:
        # [8, 128, 512] -> [8, 128, 4, 128] -> [8, 4, 128, 128] -> [G, d, d]
        gram += (
            r["gram"].astype(np.float64)
            .reshape(8, 128, 4, 128)
            .transpose(0, 2, 1, 3)
            .reshape(G, d, d)
        )

    mu16 = xh.astype(np.float64).mean(axis=0)
    W = _host_solve(gram, mu16)

    # wp[:, g*128:(g+1)*128] = W_g with partition = d (W symmetric)
    wpk = np.ascontiguousarray(
        W.transpose(1, 0, 2).reshape(d, D).astype(np.float16)
    )
    mu64 = x.mean(axis=0, dtype=np.float64)
    bvec = -np.einsum("gd,gdf->gf", mu64.reshape(G, d), W)  # [G, d]
    bbb = np.ascontiguousarray(bvec.T.astype(np.float32))  # [d, G]
    idn = np.eye(128, dtype=np.float16)

    in2 = [{"wp": wpk, "bb": bbb, "idn": idn} for _ in range(NCORES)]
    global _last_in2
    _last_in2 = in2
    r2 = run_bass_kernel_spmd(_built["k2"], in2, core_ids)
    return np.concatenate(
        [r["outT"].T.astype(np.float32) for r in r2.results], axis=0
    )


# revision 34
# speedup vs baseline: 2.1594x; 1.0155x over previous
"""GroupWhitening1d Trainium2 kernel.

x: [16384, 4096] f32, G=32 groups of d=128.
  out = (x - mean) @ blockdiag(W_g),  W_g = U_g S_g^-1/2 U_g^T from eigh of
  per-group covariance.

Strategy (data-parallel over rows, 8 cores x 2048 rows):
  K1 (device): fp16 row tiles stream from HBM on all 3 DMA rings
      (SP/Act/Pool) directly into a PERSISTENT SBUF row cache; per-group
      Gram matmuls (PE, f32 PSUM, all 8 banks) read the cache slices.
  Host: reduce grams over cores (f64), cov, eigh (f64), W; pack W_g
      blocks (fp16, partition = d) and per-feature bias b = -(mu W) as
      per-partition scalars.
  K2 (device): zero input traffic -- software-pipelined by one group:
      16 PE transposes flip the cached row-major [128,128] blocks of
      group g+1 into x^T form (f16 PSUM, [128,2048] = 2 banks) while DVE
      evacuates group g's transposes to SBUF staging and the whitening
      matmuls (W_g stationary, f32 PSUM [128,1024]) run over group g-1;
      the f32->f16 whitening evacuation adds the centering bias (DVE/Act
      split -- the only engines that can read PSUM), stores go out on
      sync/pool rings.  Host transposes out^T back and casts to f32.

  (A DMA-engine XBAR transpose-load was tried instead of PE transposes:
  concurrent XBAR streams on two HWDGE queues corrupt each other on real
  TRN2, and the tile scheduler serializes every neighboring DMA against
  an InstDmaTransposeAnt, so the XBAR path cannot be made both safe and
  fast here.)
"""

import sys
import numpy as np

if "/opt/trn_rl_repo" not in sys.path:
    sys.path.insert(0, "/opt/trn_rl_repo")

N, D, G, d = 16384, 4096, 32, 128
NCORES = 8
NS = N // NCORES  # rows per core
NT = NS // 128  # row tiles per core

_built = {}


def _sched(weights, n):
    """Deterministic weighted round-robin schedule of length n."""
    accum = dict.fromkeys(weights, 0.0)
    total = sum(weights.values())
    out = []
    for _ in range(n):
        for k in accum:
            accum[k] += weights[k] / total
        pick = max(accum, key=lambda kk: accum[kk])
        accum[pick] -= 1
        out.append(pick)
    return out


# K1 load ring per row tile / K2 store ring per group (stores stay off
# the Act ring: the Act engine is saturated by whitening evacuation)
K1_LDQ = _sched({"sync": 6, "scalar": 5, "gpsimd": 5}, NT)
K2_STQ = _sched({"gpsimd": 17, "sync": 15}, G)
# K2 whitening-evac engine per [128,1024] f32 chunk (2 per group):
# DVE also carries all transpose evacs, so Act takes most of these.
# The final group's two chunks go to different engines so the kernel
# tail evacuates in parallel.
K2_WEVAC = _sched({"s": 54, "v": 10}, 64)
K2_WEVAC[-2:] = ["v", "s"]


def _build_k1(ns=NS):
    from concourse import bacc, mybir, tile

    f16, f32 = mybir.dt.float16, mybir.dt.float32
    nc = bacc.Bacc(None, target_bir_lowering=False)
    xh = nc.dram_tensor("xh", [ns, D], f16, kind="ExternalInput")
    # layout [bank, d, gsub, e]; host: reshape/transpose to [G,d,d]
    gram = nc.dram_tensor("gram", [8, 128, 512], f32, kind="ExternalOutput")
    # persistent row cache: tile t at cols [t*D, (t+1)*D)
    cache = nc.alloc_sbuf_tensor("xrc", [128, NT * D], f16)
    with tile.TileContext(nc) as tc:
        with (
            tc.tile_pool(name="ev", bufs=8) as ev,
            tc.tile_pool(name="ps", bufs=8, space="PSUM") as ps,
        ):
            gp = [
                ps.tile([128, 512], f32, tag="gram", name=f"gram{b}")
                for b in range(8)
            ]
            # PE p-state warmup: dummy self-contained matmuls keep the PE
            # continuously busy through the first-load head so it reaches
            # full clock when the real gram stream starts (start=True
            # zeroes the bank again for the real accumulation group)
            z = ev.tile([128, 128], f16, tag="warm")
            nc.vector.memset(z[:], 0.0)
            for _ in range(60):
                nc.tensor.matmul(
                    gp[0][:, 0:1], z[:], z[:, 0:1], start=True, stop=True
                )
            for t in range(NT):
                if t == 0:
                    # split the first tile across rings so the PE can start
                    # on group 0 after ~1/4 of the tile has landed
                    for c in range(4):
                        q = (nc.sync, nc.scalar, nc.gpsimd, nc.sync)[c]
                        q.dma_start(
                            cache.ap()[:, c * 1024:(c + 1) * 1024],
                            xh[0:128, c * 1024:(c + 1) * 1024],
                        )
                else:
                    csl = cache.ap()[:, t * D:(t + 1) * D]
                    getattr(nc, K1_LDQ[t]).dma_start(
                        csl, xh[t * 128:(t + 1) * 128, :]
                    )
                for g in range(G):
                    b, s = divmod(g, 4)
                    xg = cache.ap()[:, t * D + g * 128: t * D + (g + 1) * 128]
                    # one accumulation group per PSUM bank: start zeroes the
                    # whole zero region, so only the first slice starts
                    nc.tensor.matmul(
                        gp[b][:, s * 128:(s + 1) * 128],
                        xg,
                        xg,
                        start=(t == 0 and s == 0),
                        stop=(t == NT - 1 and s == 3),
                    )
            for b in range(8):
                e = ev.tile([128, 512], f32, tag="ev")
                if b % 2 == 0:
                    nc.vector.tensor_copy(e[:], gp[b][:])
                else:
                    nc.scalar.activation(
                        e[:], gp[b][:], mybir.ActivationFunctionType.Copy
                    )
                q = (nc.sync, nc.scalar, nc.gpsimd)[b % 3]
                q.dma_start(gram[b], e[:])
    nc.compile()
    return nc


def _build_k2(ns=NS):
    from concourse import bacc, mybir, tile

    f16, f32 = mybir.dt.float16, mybir.dt.float32
    nc = bacc.Bacc(None, target_bir_lowering=False)
    # W_g stationary blocks: wp[:, g*128:(g+1)*128] = W_g (partition = d)
    wp = nc.dram_tensor("wp", [128, D], f16, kind="ExternalInput")
    idn = nc.dram_tensor("idn", [128, 128], f16, kind="ExternalInput")
    # per-feature bias as per-partition scalars: bb[f, g] = -(mu_g W_g)[f]
    bb = nc.dram_tensor("bb", [128, G], f32, kind="ExternalInput")
    # out^T: rows = feature (g*128+f), cols = n
    outT = nc.dram_tensor("outT", [D, ns], f16, kind="ExternalOutput")
    # must match _build_k1's allocation exactly (same name/shape/order)
    cache = nc.alloc_sbuf_tensor("xrc", [128, NT * D], f16)
    with tile.TileContext(nc) as tc:
        with (
            tc.tile_pool(name="cp", bufs=1) as cp,
            tc.tile_pool(name="xs", bufs=3) as xs,
            tc.tile_pool(name="st", bufs=4) as st,
            tc.tile_pool(name="pt", bufs=2, space="PSUM") as pt,
            tc.tile_pool(name="pw", bufs=3, space="PSUM") as pw,
        ):
            # identity first: the transposes only need ids + the resident
            # cache, so they start while the W chunks are still loading
            ids = cp.tile([128, 128], f16, tag="idn")
            nc.sync.dma_start(ids[:], idn[:])
            wps = cp.tile([128, D], f16, tag="wp")
            for c in range(4):
                q = (nc.sync, nc.gpsimd, nc.sync, nc.gpsimd)[c]
                q.dma_start(
                    wps[:, c * 1024:(c + 1) * 1024],
                    wp[:, c * 1024:(c + 1) * 1024],
                )
            bbs = cp.tile([128, G], f32, tag="bb")
            nc.gpsimd.dma_start(bbs[:], bb[:])

            # PE p-state warmup through the ids/W load head (see K1)
            z = cp.tile([128, 128], f16, tag="warm")
            nc.vector.memset(z[:], 0.0)
            pz = pw.tile([128, 1024], f32, tag="pw")
            for _ in range(50):
                nc.tensor.matmul(
                    pz[:, 0:1], z[:], z[:, 0:1], start=True, stop=True
                )

            # software-pipelined by one group: the PE queue is in-order, so
            # transpose(g+1) is issued before whiten(g) -- the PE works on
            # g+1's transposes while DVE stages g's x^T
            xts = {}

            def emit_xpose(g):
                # PE-transpose the 16 cached row-major [128,128] blocks of
                # this group into x^T [d, n] form, staged via f16 PSUM
                # ([128,2048] f16 = 2 banks; zero regions start per bank)
                xt = xs.tile([128, ns], f16, tag="xt")
                for hh in range(2):
                    ptile = pt.tile([128, 1024], f16, tag="pt")
                    for k in range(8):
                        t = hh * 8 + k
                        nc.tensor.matmul(
                            ptile[:, k * 128:(k + 1) * 128],
                            cache.ap()[:, t * D + g * 128:
                                       t * D + (g + 1) * 128],
                            ids[:],
                            is_transpose=True,
                            start=(k == 0),
                            stop=(k == 7),
                        )
                    # per-half evacuation starts staging the first half
                    # while the PE transposes the second
                    nc.vector.tensor_copy(
                        xt[:, hh * 1024:(hh + 1) * 1024], ptile[:]
                    )
                xts[g] = xt

            def emit_whiten(g):
                xt = xts.pop(g)
                o = st.tile([128, ns], f16, tag="st")
                bg = bbs[:, g:g + 1]
                for h in range(2):
                    p = pw.tile([128, 1024], f32, tag="pw")
                    for c in range(2):
                        cc = h * 1024 + c * 512
                        nc.tensor.matmul(
                            p[:, c * 512:(c + 1) * 512],
                            wps[:, g * 128:(g + 1) * 128],
                            xt[:, cc:cc + 512],
                            start=True,
                            stop=True,
                        )
                    # evacuation converts f32->f16 and adds the centering bias
                    sl = slice(h * 1024, (h + 1) * 1024)
                    if K2_WEVAC[2 * g + h] == "v":
                        nc.vector.tensor_scalar_add(o[:, sl], p[:], bg)
                    else:
                        nc.scalar.add(o[:, sl], p[:], bg)
                if g == G - 1:
                    # split the last store across two rings: halves the
                    # kernel's store tail
                    half = ns // 2
                    nc.sync.dma_start(
                        outT[g * 128:(g + 1) * 128, 0:half], o[:, 0:half]
                    )
                    nc.gpsimd.dma_start(
                        outT[g * 128:(g + 1) * 128, half:ns], o[:, half:ns]
                    )
                else:
                    getattr(nc, K2_STQ[g]).dma_start(
                        outT[g * 128:(g + 1) * 128, :], o[:]
                    )

            emit_xpose(0)
            for g in range(1, G):
                emit_xpose(g)
                emit_whiten(g - 1)
            emit_whiten(G - 1)
    nc.compile()
    return nc


def _sbuf_addr(nc, name):
    for a in nc.m.functions[0].allocations:
        if hasattr(a, "memorylocations") and a.memorylocations:
            ml = a.memorylocations[0]
            if ml.name == name:
                return getattr(ml, "addr", None)
    return None


def _host_solve(gram, mu):
    """gram: [G,d,d] f64 raw sum of q16(x)_g^T q16(x)_g; mu: [D] f64."""
    mug = mu.reshape(G, d)
    cov = (gram - N * np.einsum("gd,ge->gde", mug, mug)) / (N - 1)
    cov = (cov + cov.transpose(0, 2, 1)) / 2
    S, U = np.linalg.eigh(cov)
    S = np.maximum(S, 1e-12)
    W = np.einsum("gde,ge,gfe->gdf", U, 1.0 / np.sqrt(S), U)
    return W  # [G, d, d]


def kernel(x):
    from concourse.bass_utils import run_bass_kernel_spmd

    x = np.ascontiguousarray(x, dtype=np.float32)
    core_ids = list(range(NCORES))
    xh = x.astype(np.float16)

    if "k1" not in _built:
        _built["k1"] = _build_k1()
    if "k2" not in _built:
        _built["k2"] = _build_k2()
        a1 = _sbuf_addr(_built["k1"], "xrc")
        a2 = _sbuf_addr(_built["k2"], "xrc")
        assert a1 == a2 and a1 is not None, (a1, a2)

    in1 = [{"xh": xh[c * NS:(c + 1) * NS]} for c in range(NCORES)]
    r1 = run_bass_kernel_spmd(_built["k1"], in1, core_ids)
    gram = np.zeros((G, d, d), np.float64)
    for r in r1.results:
        # [8, 128, 512] -> [8, 128, 4, 128] -> [8, 4, 128, 128] -> [G, d, d]
        gram += (
            r["gram"].astype(np.float64)
            .reshape(8, 128, 4, 128)
            .transpose(0, 2, 1, 3)
            .reshape(G, d, d)
        )

    mu16 = xh.astype(np.float64).mean(axis=0)
    W = _host_solve(gram, mu16)

    # wp[:, g*128:(g+1)*128] = W_g with partition = d (W symmetric)
    wpk = np.ascontiguousarray(
        W.transpose(1, 0, 2).reshape(d, D).astype(np.float16)
    )
    mu64 = x.mean(axis=0, dtype=np.float64)
    bvec = -np.einsum("gd,gdf->gf", mu64.reshape(G, d), W)  # [G, d]
    bbb = np.ascontiguousarray(bvec.T.astype(np.float32))  # [d, G]
    idn = np.eye(128, dtype=np.float16)

    in2 = [{"wp": wpk, "bb": bbb, "idn": idn} for _ in range(NCORES)]
    global _last_in2
    _last_in2 = in2
    r2 = run_bass_kernel_spmd(_built["k2"], in2, core_ids)
    return np.concatenate(
        [r["outT"].T.astype(np.float32) for r in r2.results], axis=0
    )


# revision 35
# speedup vs baseline: 2.1691x; 1.0045x over previous
"""GroupWhitening1d Trainium2 kernel.

x: [16384, 4096] f32, G=32 groups of d=128.
  out = (x - mean) @ blockdiag(W_g),  W_g = U_g S_g^-1/2 U_g^T from eigh of
  per-group covariance.

Strategy (data-parallel over rows, 8 cores x 2048 rows):
  K1 (device): fp16 row tiles stream from HBM on all 3 DMA rings
      (SP/Act/Pool) directly into a PERSISTENT SBUF row cache; per-group
      Gram matmuls (PE, f32 PSUM, all 8 banks) read the cache slices.
  Host: reduce grams over cores (f64), cov, eigh (f64), W; pack W_g
      blocks (fp16, partition = d) and per-feature bias b = -(mu W) as
      per-partition scalars.
  K2 (device): zero input traffic -- software-pipelined by one group:
      16 PE transposes flip the cached row-major [128,128] blocks of
      group g+1 into x^T form (f16 PSUM, [128,2048] = 2 banks) while DVE
      evacuates group g's transposes to SBUF staging and the whitening
      matmuls (W_g stationary, f32 PSUM [128,1024]) run over group g-1;
      the f32->f16 whitening evacuation adds the centering bias (DVE/Act
      split -- the only engines that can read PSUM), stores go out on
      sync/pool rings.  Host transposes out^T back and casts to f32.

  (A DMA-engine XBAR transpose-load was tried instead of PE transposes:
  concurrent XBAR streams on two HWDGE queues corrupt each other on real
  TRN2, and the tile scheduler serializes every neighboring DMA against
  an InstDmaTransposeAnt, so the XBAR path cannot be made both safe and
  fast here.)
"""

import sys
import numpy as np

if "/opt/trn_rl_repo" not in sys.path:
    sys.path.insert(0, "/opt/trn_rl_repo")

N, D, G, d = 16384, 4096, 32, 128
NCORES = 8
NS = N // NCORES  # rows per core
NT = NS // 128  # row tiles per core

_built = {}


def _sched(weights, n):
    """Deterministic weighted round-robin schedule of length n."""
    accum = dict.fromkeys(weights, 0.0)
    total = sum(weights.values())
    out = []
    for _ in range(n):
        for k in accum:
            accum[k] += weights[k] / total
        pick = max(accum, key=lambda kk: accum[kk])
        accum[pick] -= 1
        out.append(pick)
    return out


# K1 load ring per row tile / K2 store ring per group (stores stay off
# the Act ring: the Act engine is saturated by whitening evacuation)
K1_LDQ = _sched({"sync": 6, "scalar": 5, "gpsimd": 5}, NT)
K2_STQ = _sched({"gpsimd": 17, "sync": 15}, G)
# K2 whitening-evac engine per [128,1024] f32 chunk (2 per group):
# DVE also carries all transpose evacs, so Act takes most of these.
# The final group's two chunks go to different engines so the kernel
# tail evacuates in parallel.
K2_WEVAC = _sched({"s": 54, "v": 10}, 64)
K2_WEVAC[-2:] = ["v", "s"]


def _build_k1(ns=NS):
    from concourse import bacc, mybir, tile

    f16, f32 = mybir.dt.float16, mybir.dt.float32
    nc = bacc.Bacc(None, target_bir_lowering=False)
    xh = nc.dram_tensor("xh", [ns, D], f16, kind="ExternalInput")
    # layout [bank, d, gsub, e]; host: reshape/transpose to [G,d,d]
    gram = nc.dram_tensor("gram", [8, 128, 512], f16, kind="ExternalOutput")
    # persistent row cache: tile t at cols [t*D, (t+1)*D)
    cache = nc.alloc_sbuf_tensor("xrc", [128, NT * D], f16)
    with tile.TileContext(nc) as tc:
        with (
            tc.tile_pool(name="ev", bufs=8) as ev,
            tc.tile_pool(name="ps", bufs=8, space="PSUM") as ps,
        ):
            gp = [
                ps.tile([128, 512], f32, tag="gram", name=f"gram{b}")
                for b in range(8)
            ]
            # PE p-state warmup: dummy self-contained matmuls keep the PE
            # continuously busy through the first-load head so it reaches
            # full clock when the real gram stream starts (start=True
            # zeroes the bank again for the real accumulation group)
            z = ev.tile([128, 128], f16, tag="warm")
            nc.vector.memset(z[:], 0.0)
            for _ in range(60):
                nc.tensor.matmul(
                    gp[0][:, 0:1], z[:], z[:, 0:1], start=True, stop=True
                )
            for t in range(NT):
                if t == 0:
                    # split the first tile across rings so the PE can start
                    # on group 0 after ~1/4 of the tile has landed
                    for c in range(4):
                        q = (nc.sync, nc.scalar, nc.gpsimd, nc.sync)[c]
                        q.dma_start(
                            cache.ap()[:, c * 1024:(c + 1) * 1024],
                            xh[0:128, c * 1024:(c + 1) * 1024],
                        )
                else:
                    csl = cache.ap()[:, t * D:(t + 1) * D]
                    getattr(nc, K1_LDQ[t]).dma_start(
                        csl, xh[t * 128:(t + 1) * 128, :]
                    )
                for g in range(G):
                    b, s = divmod(g, 4)
                    xg = cache.ap()[:, t * D + g * 128: t * D + (g + 1) * 128]
                    # one accumulation group per PSUM bank: start zeroes the
                    # whole zero region, so only the first slice starts
                    nc.tensor.matmul(
                        gp[b][:, s * 128:(s + 1) * 128],
                        xg,
                        xg,
                        start=(t == 0 and s == 0),
                        stop=(t == NT - 1 and s == 3),
                    )
            for b in range(8):
                e = ev.tile([128, 512], f16, tag="ev")
                if b % 2 == 0:
                    nc.vector.tensor_copy(e[:], gp[b][:])
                else:
                    nc.scalar.activation(
                        e[:], gp[b][:], mybir.ActivationFunctionType.Copy
                    )
                if b == 7:
                    # split the final bank's store so the kernel tail
                    # overlaps two rings
                    nc.sync.dma_start(gram[b][:, 0:256], e[:, 0:256])
                    nc.gpsimd.dma_start(gram[b][:, 256:512], e[:, 256:512])
                else:
                    q = (nc.sync, nc.scalar, nc.gpsimd)[b % 3]
                    q.dma_start(gram[b], e[:])
    nc.compile()
    return nc


def _build_k2(ns=NS):
    from concourse import bacc, mybir, tile

    f16, f32 = mybir.dt.float16, mybir.dt.float32
    nc = bacc.Bacc(None, target_bir_lowering=False)
    # W_g stationary blocks: wp[:, g*128:(g+1)*128] = W_g (partition = d)
    wp = nc.dram_tensor("wp", [128, D], f16, kind="ExternalInput")
    idn = nc.dram_tensor("idn", [128, 128], f16, kind="ExternalInput")
    # per-feature bias as per-partition scalars: bb[f, g] = -(mu_g W_g)[f]
    bb = nc.dram_tensor("bb", [128, G], f32, kind="ExternalInput")
    # out^T: rows = feature (g*128+f), cols = n
    outT = nc.dram_tensor("outT", [D, ns], f16, kind="ExternalOutput")
    # must match _build_k1's allocation exactly (same name/shape/order)
    cache = nc.alloc_sbuf_tensor("xrc", [128, NT * D], f16)
    with tile.TileContext(nc) as tc:
        with (
            tc.tile_pool(name="cp", bufs=1) as cp,
            tc.tile_pool(name="xs", bufs=3) as xs,
            tc.tile_pool(name="st", bufs=4) as st,
            tc.tile_pool(name="pt", bufs=2, space="PSUM") as pt,
            tc.tile_pool(name="pw", bufs=3, space="PSUM") as pw,
        ):
            # identity first: the transposes only need ids + the resident
            # cache, so they start while the W chunks are still loading
            ids = cp.tile([128, 128], f16, tag="idn")
            nc.sync.dma_start(ids[:], idn[:])
            wps = cp.tile([128, D], f16, tag="wp")
            for c in range(4):
                q = (nc.sync, nc.gpsimd, nc.sync, nc.gpsimd)[c]
                q.dma_start(
                    wps[:, c * 1024:(c + 1) * 1024],
                    wp[:, c * 1024:(c + 1) * 1024],
                )
            bbs = cp.tile([128, G], f32, tag="bb")
            nc.gpsimd.dma_start(bbs[:], bb[:])

            # PE p-state warmup through the ids/W load head (see K1)
            z = cp.tile([128, 128], f16, tag="warm")
            nc.vector.memset(z[:], 0.0)
            pz = pw.tile([128, 1024], f32, tag="pw")
            for _ in range(50):
                nc.tensor.matmul(
                    pz[:, 0:1], z[:], z[:, 0:1], start=True, stop=True
                )

            # software-pipelined by one group: the PE queue is in-order, so
            # transpose(g+1) is issued before whiten(g) -- the PE works on
            # g+1's transposes while DVE stages g's x^T
            xts = {}

            def emit_xpose(g):
                # PE-transpose the 16 cached row-major [128,128] blocks of
                # this group into x^T [d, n] form, staged via f16 PSUM
                # ([128,2048] f16 = 2 banks; zero regions start per bank)
                xt = xs.tile([128, ns], f16, tag="xt")
                for hh in range(2):
                    ptile = pt.tile([128, 1024], f16, tag="pt")
                    for k in range(8):
                        t = hh * 8 + k
                        nc.tensor.matmul(
                            ptile[:, k * 128:(k + 1) * 128],
                            cache.ap()[:, t * D + g * 128:
                                       t * D + (g + 1) * 128],
                            ids[:],
                            is_transpose=True,
                            start=(k == 0),
                            stop=(k == 7),
                        )
                    # per-half evacuation starts staging the first half
                    # while the PE transposes the second
                    nc.vector.tensor_copy(
                        xt[:, hh * 1024:(hh + 1) * 1024], ptile[:]
                    )
                xts[g] = xt

            def emit_whiten(g):
                xt = xts.pop(g)
                o = st.tile([128, ns], f16, tag="st")
                bg = bbs[:, g:g + 1]
                for h in range(2):
                    p = pw.tile([128, 1024], f32, tag="pw")
                    for c in range(2):
                        cc = h * 1024 + c * 512
                        nc.tensor.matmul(
                            p[:, c * 512:(c + 1) * 512],
                            wps[:, g * 128:(g + 1) * 128],
                            xt[:, cc:cc + 512],
                            start=True,
                            stop=True,
                        )
                    # evacuation converts f32->f16 and adds the centering bias
                    sl = slice(h * 1024, (h + 1) * 1024)
                    if K2_WEVAC[2 * g + h] == "v":
                        nc.vector.tensor_scalar_add(o[:, sl], p[:], bg)
                    else:
                        nc.scalar.add(o[:, sl], p[:], bg)
                if g == G - 1:
                    # split the last store across two rings: halves the
                    # kernel's store tail
                    half = ns // 2
                    nc.sync.dma_start(
                        outT[g * 128:(g + 1) * 128, 0:half], o[:, 0:half]
                    )
                    nc.gpsimd.dma_start(
                        outT[g * 128:(g + 1) * 128, half:ns], o[:, half:ns]
                    )
                else:
                    getattr(nc, K2_STQ[g]).dma_start(
                        outT[g * 128:(g + 1) * 128, :], o[:]
                    )

            emit_xpose(0)
            for g in range(1, G):
                emit_xpose(g)
                emit_whiten(g - 1)
            emit_whiten(G - 1)
    nc.compile()
    return nc


def _sbuf_addr(nc, name):
    for a in nc.m.functions[0].allocations:
        if hasattr(a, "memorylocations") and a.memorylocations:
            ml = a.memorylocations[0]
            if ml.name == name:
                return getattr(ml, "addr", None)
    return None


def _host_solve(gram, mu):
    """gram: [G,d,d] f64 raw sum of q16(x)_g^T q16(x)_g; mu: [D] f64."""
    mug = mu.reshape(G, d)
    cov = (gram - N * np.einsum("gd,ge->gde", mug, mug)) / (N - 1)
    cov = (cov + cov.transpose(0, 2, 1)) / 2
    S, U = np.linalg.eigh(cov)
    S = np.maximum(S, 1e-12)
    W = np.einsum("gde,ge,gfe->gdf", U, 1.0 / np.sqrt(S), U)
    return W  # [G, d, d]


def kernel(x):
    from concourse.bass_utils import run_bass_kernel_spmd

    x = np.ascontiguousarray(x, dtype=np.float32)
    core_ids = list(range(NCORES))
    xh = x.astype(np.float16)

    if "k1" not in _built:
        _built["k1"] = _build_k1()
    if "k2" not in _built:
        _built["k2"] = _build_k2()
        a1 = _sbuf_addr(_built["k1"], "xrc")
        a2 = _sbuf_addr(_built["k2"], "xrc")
        assert a1 == a2 and a1 is not None, (a1, a2)

    in1 = [{"xh": xh[c * NS:(c + 1) * NS]} for c in range(NCORES)]
    r1 = run_bass_kernel_spmd(_built["k1"], in1, core_ids)
    gram = np.zeros((G, d, d), np.float64)
    for r in r1.results:
        # [8, 128, 512] -> [8, 128, 4, 128] -> [8, 4, 128, 128] -> [G, d, d]
        gram += (
            r["gram"].astype(np.float64)
            .reshape(8, 128, 4, 128)
            .transpose(0, 2, 1, 3)
            .reshape(G, d, d)
        )

    mu16 = xh.astype(np.float64).mean(axis=0)
    W = _host_solve(gram, mu16)

    # wp[:, g*128:(g+1)*128] = W_g with partition = d (W symmetric)
    wpk = np.ascontiguousarray(
        W.transpose(1, 0, 2).reshape(d, D).astype(np.float16)
    )
    mu64 = x.mean(axis=0, dtype=np.float64)
    bvec = -np.einsum("gd,gdf->gf", mu64.reshape(G, d), W)  # [G, d]
    bbb = np.ascontiguousarray(bvec.T.astype(np.float32))  # [d, G]
    idn = np.eye(128, dtype=np.float16)

    in2 = [{"wp": wpk, "bb": bbb, "idn": idn} for _ in range(NCORES)]
    global _last_in2
    _last_in2 = in2
    r2 = run_bass_kernel_spmd(_built["k2"], in2, core_ids)
    return np.concatenate(
        [r["outT"].T.astype(np.float32) for r in r2.results], axis=0
    )
